# revision 6
# baseline (speedup 1.0000x reference)
"""GroupWiseTemporalAttention Trainium2 kernel.

Math: in the reference, SDPA runs with seq-len L=S=1 per channel-group, so
softmax over the single key is identically 1 and the attention output equals
v = (x+pe)_group @ v_w.T + v_b.  The whole module therefore folds into one
affine map:

    out = x_flat @ W_eff + b_eff
    W_eff = kron(I_192, v_w.T) @ proj_w.T            # [768, 768]
    b_eff = pe@W_eff + tile(v_b,192)@proj_w.T + proj_b

which we run as a data-parallel GEMM over 8 NeuronCores (6272 rows each).
The per-core kernel streams pre-transposed x^T tiles as the stationary
matmul operand so output lands in natural [tokens, channels] layout.
"""

import os

import numpy as np
import ml_dtypes

import concourse.bass as bass
import concourse.mybir as mybir
import concourse.tile as tile
from concourse import bacc
from concourse.bass_utils import run_bass_kernel_spmd

P = 128
C = 768
KC = C // P            # 6 contraction chunks
N_CORES = 8
B, H, W = 16, 56, 56
ROWS = B * H * W       # 50176
RPC = ROWS // N_CORES  # 6272 rows per core
TT = RPC // P          # 49 token tiles per core
TBLK = 4               # token tiles per input DMA block (512 tokens)
N_WARM = 8             # PE pre-warm matmuls issued during the DMA head
OUT_BF16 = os.environ.get("GWTA_OUT", "bf16") == "bf16"

# Internal matmul dtype: bf16 halves input DMA and streams 1 col/cycle.
# fp32r keeps fp32 storage (full DMA) at 1 col/cycle for free-dim>=256.
VARIANT = os.environ.get("GWTA_VARIANT", "bf16")

LAST_STATS: dict = {}

_IN_DT = {
    "bf16": mybir.dt.bfloat16,
    "fp32r": mybir.dt.float32r,
    "fp32": mybir.dt.float32,
}


def _build_nc(variant: str) -> bass.Bass:
    in_dt = _IN_DT[variant]
    nc = bacc.Bacc(None, target_bir_lowering=False)
    out_dt = mybir.dt.bfloat16 if OUT_BF16 else mybir.dt.float32
    xT = nc.declare_dram_parameter("xT", [C, RPC], in_dt, isOutput=False)
    w = nc.declare_dram_parameter("w", [C, C], in_dt, isOutput=False)
    b = nc.declare_dram_parameter("b", [P, C], mybir.dt.float32, isOutput=False)
    out = nc.declare_dram_parameter(
        "out", [RPC, C], out_dt, isOutput=True
    )

    with tile.TileContext(nc) as tc:
        with (
            tc.tile_pool(name="const", bufs=1) as const,
            tc.tile_pool(name="xp", bufs=3) as xp,
            tc.tile_pool(name="op", bufs=4) as op,
            tc.tile_pool(name="pp", bufs=1, space="PSUM") as pp,
        ):
            # PE pre-warm: matmuls on zeroed SBUF keep the PE busy from the
            # end of the NEFF preamble (~6.5us) until the first weight/x
            # tiles land (~9.4us), so the HAM activity window accumulates
            # continuous busy time and un-throttles to 2.4GHz right as the
            # real stream begins.  GpSimd does the memset because its queue
            # is free ~0.7us before DVE's during the preamble.  The warm
            # matmuls borrow psum slot "pt3", which the real stream touches
            # last.
            g_rhs = const.tile([P, 512], in_dt)
            nc.gpsimd.memset(g_rhs[:], 0.0)
            warm = pp.tile([P, C], mybir.dt.float32, tag="pt3")
            for _ in range(N_WARM):
                nc.tensor.matmul(
                    warm[:, 0:512], g_rhs[:, 0:P], g_rhs[:],
                    start=True, stop=True,
                )

            xTr = xT.rearrange("(kc p) t -> p kc t", p=P)

            # Weights resident in SBUF, one tile per contraction chunk, and
            # block 0's x chunks interleaved with them so the first real
            # matmul pair (w0, x0) waits on the minimum number of bytes.
            # Block-0 inputs go out on gpsimd's SWDGE ring: gpsimd's code
            # lands ~2us before the HWDGE sequencers start issuing, so the
            # critical first bytes arrive earlier.
            wts = []
            xts0 = []
            for kc in range(KC):
                wt = const.tile([P, C], in_dt, tag=f"w{kc}", name=f"w{kc}")
                nc.sync.dma_start(
                    out=wt[:], in_=w[kc * P : (kc + 1) * P, :]
                )
                wts.append(wt)
                xt = xp.tile([P, P], in_dt, tag=f"x{kc}", name=f"x{kc}0")
                nc.sync.dma_start(out=xt[:], in_=xTr[:, kc, 0:P])
                xts0.append(xt)
            # 49 token tiles: a 1-tile block first (so the critical head
            # data is just W + 6 small x chunks), then 12 4-tile blocks.
            blocks = [1] + [TBLK] * ((TT - 1) // TBLK)
            assert sum(blocks) == TT

            # Block 1's x rides the scalar ring BEFORE the bias: it gates
            # the PE at ~12us while the bias is first read by DVE ~2us
            # later.
            xt1 = xp.tile([P, KC, TBLK * P], in_dt, tag="xb", name="xb1")
            nc.scalar.dma_start(
                out=xt1[:, :, : blocks[1] * P],
                in_=xTr[:, :, blocks[0] * P : (blocks[0] + blocks[1]) * P],
            )
            # Bias replicated across partitions (host provides [128, C]).
            bt = const.tile([P, C], mybir.dt.float32)
            nc.scalar.dma_start(out=bt[:], in_=b[:])

            t0 = 0
            for bi, nb in enumerate(blocks):
                if bi == 0:
                    def xslice(kc, s):
                        return xts0[kc][:, s * P : (s + 1) * P]
                else:
                    if bi == 1:
                        xt = xt1
                    else:
                        xt = xp.tile(
                            [P, KC, TBLK * P], in_dt, tag="xb", name="xb"
                        )
                        nc.scalar.dma_start(
                            out=xt[:, :, : nb * P],
                            in_=xTr[:, :, t0 * P : (t0 + nb) * P],
                        )

                    def xslice(kc, s, xt=xt):
                        return xt[:, kc, s * P : (s + 1) * P]

                pts = [
                    pp.tile(
                        [P, C], mybir.dt.float32, tag=f"pt{s}", name=f"pt{s}"
                    )
                    for s in range(nb)
                ]
                # s-outer: each psum group completes a quarter-block ahead
                # of the next, so bias-add TTs overlap the matmul stream.
                for s in range(nb):
                    for kc in range(KC):
                        lhsT = xslice(kc, s)
                        nc.tensor.matmul(
                            pts[s][:, 0:512], lhsT, wts[kc][:, 0:512],
                            start=(kc == 0), stop=(kc == KC - 1),
                        )
                        nc.tensor.matmul(
                            pts[s][:, 512:C], lhsT, wts[kc][:, 512:C],
                            start=(kc == 0), stop=(kc == KC - 1),
                        )
                for s in range(nb):
                    ot = op.tile([P, C], out_dt, tag="ot")
                    # split at the PSUM bank boundary (one bank per DVE read)
                    nc.vector.tensor_add(
                        out=ot[:, 0:512], in0=pts[s][:, 0:512], in1=bt[:, 0:512]
                    )
                    nc.vector.tensor_add(
                        out=ot[:, 512:C], in0=pts[s][:, 512:C], in1=bt[:, 512:C]
                    )
                    nc.sync.dma_start(
                        out=out[(t0 + s) * P : (t0 + s + 1) * P, :], in_=ot[:]
                    )
                t0 += nb
    nc.compile()
    return nc


def _fold_weights(qkv_w, qkv_b, proj_w, proj_b, pe):
    v_w = qkv_w[2 * 4 : 3 * 4].astype(np.float64)   # [4, 4]
    v_b = qkv_b[2 * 4 : 3 * 4].astype(np.float64)   # [4]
    bd = np.kron(np.eye(C // 4), v_w.T)             # y_flat @ bd == groupwise v
    w_eff = bd @ proj_w.astype(np.float64).T        # [768, 768]
    b_eff = (
        np.tile(v_b, C // 4) @ proj_w.astype(np.float64).T
        + proj_b.astype(np.float64)
        + pe[:C].astype(np.float64) @ w_eff
    )
    return w_eff, b_eff


def _enable_tracing_shims():
    """Dev-only (GWTA_TRACE=1): restore the NTFF profile hook that this
    image's `antenv` is missing, and keep trace artifacts local instead of
    uploading.  Never active when the kernel is called normally."""
    import sys
    import types

    try:
        from antenv import axon_hooks  # noqa: F401
    except ImportError:
        import antenv
        from trn_agent_boot.trn_boot import _ntff_profile_via_ctypes

        mod = types.ModuleType("antenv.axon_hooks")
        mod._hook = _ntff_profile_via_ctypes("/opt/axon/libaxon_pjrt.so")
        mod.get_axon_ntff_profile_hook = lambda: mod._hook
        mod.set_axon_ntff_profile_hook = lambda h: setattr(mod, "_hook", h)
        sys.modules["antenv.axon_hooks"] = mod
        antenv.axon_hooks = mod

    import concourse.bass_utils as bu

    bu.upload_artifacts = lambda tmpdir: f"local:{tmpdir}"


def kernel(x, qkv_w, qkv_b, proj_w, proj_b, pe):
    x = np.asarray(x, np.float32)
    w_eff, b_eff = _fold_weights(
        np.asarray(qkv_w), np.asarray(qkv_b),
        np.asarray(proj_w), np.asarray(proj_b), np.asarray(pe),
    )

    variant = VARIANT
    if variant == "bf16":
        cast = lambda a: np.ascontiguousarray(a, dtype=ml_dtypes.bfloat16)
    else:
        cast = lambda a: np.ascontiguousarray(a, dtype=np.float32)

    w_dev = cast(w_eff)
    b_dev = np.broadcast_to(
        b_eff.astype(np.float32), (P, C)
    ).copy()

    x_flat = x.reshape(ROWS, C)
    in_maps = []
    for c in range(N_CORES):
        xt = cast(x_flat[c * RPC : (c + 1) * RPC].T)
        in_maps.append({"xT": xt, "w": w_dev, "b": b_dev})

    nc = _build_nc(variant)
    trace = bool(int(os.environ.get("GWTA_TRACE", "0")))
    kw = {}
    if trace:
        _enable_tracing_shims()
        kw["tmpdir"] = os.environ.get("GWTA_TRACE_DIR") or None
    r = run_bass_kernel_spmd(nc, in_maps, list(range(N_CORES)), trace=trace, **kw)

    LAST_STATS.clear()
    LAST_STATS.update(
        exec_time_ns=r.exec_time_ns,
        mean_exec_time_ns=r.mean_exec_time_ns,
        variant=variant,
    )

    out = np.empty((ROWS, C), np.float32)
    for c in range(N_CORES):
        out[c * RPC : (c + 1) * RPC] = np.asarray(
            r.results[c]["out"]
        ).astype(np.float32)
    return out.reshape(B, H, W, C)



# revision 7
# speedup vs baseline: 1.0536x; 1.0536x over previous
"""GroupWiseTemporalAttention Trainium2 kernel.

Math: in the reference, SDPA runs with seq-len L=S=1 per channel-group, so
softmax over the single key is identically 1 and the attention output equals
v = (x+pe)_group @ v_w.T + v_b.  The whole module therefore folds into one
affine map:

    out = x_flat @ W_eff + b_eff
    W_eff = kron(I_192, v_w.T) @ proj_w.T            # [768, 768]
    b_eff = pe@W_eff + tile(v_b,192)@proj_w.T + proj_b

which we run as a data-parallel GEMM over 8 NeuronCores (6272 rows each).
The per-core kernel streams pre-transposed x^T tiles as the stationary
matmul operand so output lands in natural [tokens, channels] layout.

Head schedule: every HWDGE dma_start costs ~0.65us of sequencer issue
time, so the critical head data (weights + first token block) is fused
host-side into one [128, 6, 1280] tensor and fetched with just six
per-kc DMAs on the sync ring.  Block 0 runs kc-outer so matmul pass k
only needs head chunk k -- the PE streams gaplessly from the first
chunk's arrival instead of waiting for the full weight matrix.  All
input DMAs ride the sync ring in consumption order (single FIFO queue
=> bytes arrive in priority order); output DMAs ride the scalar ring.
"""

import os

import numpy as np
import ml_dtypes

import concourse.bass as bass
import concourse.mybir as mybir
import concourse.tile as tile
from concourse import bacc
from concourse.bass_utils import run_bass_kernel_spmd

P = 128
C = 768
KC = C // P            # 6 contraction chunks
N_CORES = 8
B, H, W = 16, 56, 56
ROWS = B * H * W       # 50176
RPC = ROWS // N_CORES  # 6272 rows per core
TT = RPC // P          # 49 token tiles per core
TBLK = 4               # token tiles per input DMA block (512 tokens)
HBLK = 4               # token tiles in the fused head block
HW_ = C + HBLK * P     # head chunk cols per kc: 768 w + 512 x0
N_WARM = 7             # PE pre-warm matmuls issued during the DMA head
OUT_BF16 = os.environ.get("GWTA_OUT", "bf16") == "bf16"

# Internal matmul dtype: bf16 halves input DMA and streams 1 col/cycle.
VARIANT = os.environ.get("GWTA_VARIANT", "bf16")

LAST_STATS: dict = {}

_IN_DT = {
    "bf16": mybir.dt.bfloat16,
    "fp32r": mybir.dt.float32r,
    "fp32": mybir.dt.float32,
}


def _build_nc(variant: str) -> bass.Bass:
    in_dt = _IN_DT[variant]
    out_dt = mybir.dt.bfloat16 if OUT_BF16 else mybir.dt.float32
    nc = bacc.Bacc(None, target_bir_lowering=False)
    hx = nc.declare_dram_parameter("hx", [P, KC, HW_], in_dt, isOutput=False)
    xT = nc.declare_dram_parameter("xT", [C, RPC], in_dt, isOutput=False)
    b = nc.declare_dram_parameter("b", [P, C], mybir.dt.float32, isOutput=False)
    out = nc.declare_dram_parameter("out", [RPC, C], out_dt, isOutput=True)

    with tile.TileContext(nc) as tc:
        with (
            tc.tile_pool(name="const", bufs=1) as const,
            tc.tile_pool(name="xp", bufs=3) as xp,
            tc.tile_pool(name="op", bufs=4) as op,
            tc.tile_pool(name="pp", bufs=1, space="PSUM") as pp,
        ):
            # PE pre-warm: matmuls on zeroed SBUF keep the PE busy from the
            # end of the NEFF preamble (~6.5us) until the first head chunk
            # lands (~9us), so the HAM activity window accumulates
            # continuous busy time and un-throttles to 2.4GHz shortly after
            # the real stream begins.  GpSimd does the memset because its
            # queue drains ~0.7us before DVE's during the preamble.
            g_rhs = const.tile([P, 512], in_dt)
            nc.gpsimd.memset(g_rhs[:], 0.0)
            warm = pp.tile([P, C], mybir.dt.float32, tag="pt3")
            for _ in range(N_WARM):
                nc.tensor.matmul(
                    warm[:, 0:512], g_rhs[:, 0:P], g_rhs[:],
                    start=True, stop=True,
                )

            xTr = xT.rearrange("(kc p) t -> p kc t", p=P)

            # Head chunks: [w_kc | x0_kc] fused, one DMA per kc on the sync
            # ring.  Pass kc of block 0 needs only chunk kc.
            ht = const.tile([P, KC, HW_], in_dt, name="ht")
            for kc in range(KC):
                nc.sync.dma_start(out=ht[:, kc, :], in_=hx[:, kc, :])
            # Bias (first read by DVE ~15us), then 2 prefetched x blocks --
            # all behind the head chunks on the same FIFO queue.
            bt = const.tile([P, C], mybir.dt.float32)
            nc.sync.dma_start(out=bt[:], in_=b[:])

            blocks = [TBLK] * ((TT - HBLK) // TBLK) + [1]
            assert HBLK + sum(blocks) == TT

            xbts: dict = {}

            def issue_xb(j: int) -> None:
                nbj = blocks[j]
                t0j = HBLK + sum(blocks[:j])
                xt = xp.tile([P, KC, TBLK * P], in_dt, tag="xb", name="xb")
                nc.sync.dma_start(
                    out=xt[:, :, : nbj * P],
                    in_=xTr[:, :, t0j * P : (t0j + nbj) * P],
                )
                xbts[j] = xt

            issue_xb(0)
            issue_xb(1)

            def wslice(kc, half):
                return (
                    ht[:, kc, 0:512] if half == 0 else ht[:, kc, 512:C]
                )

            def evict(g: int, pt) -> None:
                ot = op.tile([P, C], out_dt, tag="ot")
                # split at the PSUM bank boundary (one bank per DVE read)
                nc.vector.tensor_add(
                    out=ot[:, 0:512], in0=pt[:, 0:512], in1=bt[:, 0:512]
                )
                nc.vector.tensor_add(
                    out=ot[:, 512:C], in0=pt[:, 512:C], in1=bt[:, 512:C]
                )
                nc.scalar.dma_start(
                    out=out[g * P : (g + 1) * P, :], in_=ot[:]
                )

            # Block 0: kc-outer so pass kc only needs head chunk kc.  The
            # kc=5 pass runs s-ascending, so psum tile s=0 completes first
            # and its eviction clears tag pt0 just before block 1 reuses it.
            pts = [
                pp.tile([P, C], mybir.dt.float32, tag=f"pt{s}", name=f"pt{s}")
                for s in range(HBLK)
            ]
            for kc in range(KC):
                for s in range(HBLK):
                    lhsT = ht[:, kc, C + s * P : C + (s + 1) * P]
                    nc.tensor.matmul(
                        pts[s][:, 0:512], lhsT, wslice(kc, 0),
                        start=(kc == 0), stop=(kc == KC - 1),
                    )
                    nc.tensor.matmul(
                        pts[s][:, 512:C], lhsT, wslice(kc, 1),
                        start=(kc == 0), stop=(kc == KC - 1),
                    )
            for s in range(HBLK):
                evict(s, pts[s])

            # Blocks 1..: s-outer, one 4-tile input DMA each, prefetch
            # distance 2.
            g0 = HBLK
            for bi, nb in enumerate(blocks):
                if bi + 2 < len(blocks):
                    issue_xb(bi + 2)
                xt = xbts.pop(bi)
                pts = [
                    pp.tile(
                        [P, C], mybir.dt.float32,
                        tag=f"pt{(g0 + s) % 4}", name=f"pt{(g0 + s) % 4}",
                    )
                    for s in range(nb)
                ]
                for s in range(nb):
                    for kc in range(KC):
                        lhsT = xt[:, kc, s * P : (s + 1) * P]
                        nc.tensor.matmul(
                            pts[s][:, 0:512], lhsT, wslice(kc, 0),
                            start=(kc == 0), stop=(kc == KC - 1),
                        )
                        nc.tensor.matmul(
                            pts[s][:, 512:C], lhsT, wslice(kc, 1),
                            start=(kc == 0), stop=(kc == KC - 1),
                        )
                for s in range(nb):
                    evict(g0 + s, pts[s])
                g0 += nb
    nc.compile()
    return nc


def _fold_weights(qkv_w, qkv_b, proj_w, proj_b, pe):
    v_w = qkv_w[2 * 4 : 3 * 4].astype(np.float64)   # [4, 4]
    v_b = qkv_b[2 * 4 : 3 * 4].astype(np.float64)   # [4]
    bd = np.kron(np.eye(C // 4), v_w.T)             # y_flat @ bd == groupwise v
    w_eff = bd @ proj_w.astype(np.float64).T        # [768, 768]
    b_eff = (
        np.tile(v_b, C // 4) @ proj_w.astype(np.float64).T
        + proj_b.astype(np.float64)
        + pe[:C].astype(np.float64) @ w_eff
    )
    return w_eff, b_eff


def _enable_tracing_shims():
    """Dev-only (GWTA_TRACE=1): restore the NTFF profile hook that this
    image's `antenv` is missing, and keep trace artifacts local instead of
    uploading.  Never active when the kernel is called normally."""
    import sys
    import types

    try:
        from antenv import axon_hooks  # noqa: F401
    except ImportError:
        import antenv
        from trn_agent_boot.trn_boot import _ntff_profile_via_ctypes

        mod = types.ModuleType("antenv.axon_hooks")
        mod._hook = _ntff_profile_via_ctypes("/opt/axon/libaxon_pjrt.so")
        mod.get_axon_ntff_profile_hook = lambda: mod._hook
        mod.set_axon_ntff_profile_hook = lambda h: setattr(mod, "_hook", h)
        sys.modules["antenv.axon_hooks"] = mod
        antenv.axon_hooks = mod

    import concourse.bass_utils as bu

    bu.upload_artifacts = lambda tmpdir: f"local:{tmpdir}"


def kernel(x, qkv_w, qkv_b, proj_w, proj_b, pe):
    x = np.asarray(x, np.float32)
    w_eff, b_eff = _fold_weights(
        np.asarray(qkv_w), np.asarray(qkv_b),
        np.asarray(proj_w), np.asarray(proj_b), np.asarray(pe),
    )

    variant = VARIANT
    if variant == "bf16":
        cast = lambda a: np.ascontiguousarray(a, dtype=ml_dtypes.bfloat16)
    else:
        cast = lambda a: np.ascontiguousarray(a, dtype=np.float32)

    w_dev = cast(w_eff)                       # [768, 768]
    w_chunks = np.asarray(w_dev).reshape(KC, P, C)  # [kc, p, c]
    b_dev = np.broadcast_to(b_eff.astype(np.float32), (P, C)).copy()

    x_flat = x.reshape(ROWS, C)
    in_maps = []
    for c in range(N_CORES):
        xc = x_flat[c * RPC : (c + 1) * RPC]
        xt = cast(xc.T)                       # [768, 6272]
        xt_np = np.asarray(xt)
        # Fused head: [p, kc, 768 w | 512 x0]
        hx = np.empty((P, KC, HW_), dtype=xt_np.dtype)
        hx[:, :, :C] = w_chunks.transpose(1, 0, 2)
        hx[:, :, C:] = (
            xt_np[:, : HBLK * P].reshape(KC, P, HBLK * P).transpose(1, 0, 2)
        )
        in_maps.append({"hx": hx, "xT": xt_np, "b": b_dev})

    nc = _build_nc(variant)
    trace = bool(int(os.environ.get("GWTA_TRACE", "0")))
    kw = {}
    if trace:
        _enable_tracing_shims()
        kw["tmpdir"] = os.environ.get("GWTA_TRACE_DIR") or None
    r = run_bass_kernel_spmd(nc, in_maps, list(range(N_CORES)), trace=trace, **kw)

    LAST_STATS.clear()
    LAST_STATS.update(
        exec_time_ns=r.exec_time_ns,
        mean_exec_time_ns=r.mean_exec_time_ns,
        variant=variant,
    )

    out = np.empty((ROWS, C), np.float32)
    for c in range(N_CORES):
        out[c * RPC : (c + 1) * RPC] = np.asarray(
            r.results[c]["out"]
        ).astype(np.float32)
    return out.reshape(B, H, W, C)


# revision 12
# speedup vs baseline: 1.0633x; 1.0092x over previous
"""GroupWiseTemporalAttention Trainium2 kernel.

Math: in the reference, SDPA runs with seq-len L=S=1 per channel-group, so
softmax over the single key is identically 1 and the attention output equals
v = (x+pe)_group @ v_w.T + v_b.  The whole module therefore folds into one
affine map:

    out = x_flat @ W_eff + b_eff
    W_eff = kron(I_192, v_w.T) @ proj_w.T            # [768, 768]
    b_eff = pe@W_eff + tile(v_b,192)@proj_w.T + proj_b

which we run as a data-parallel GEMM over 8 NeuronCores (6272 rows each).
The per-core kernel streams pre-transposed x^T tiles as the stationary
matmul operand so output lands in natural [tokens, channels] layout.

Head schedule: every HWDGE dma_start costs ~0.65us of sequencer issue
time, so the critical head data (weights + first token block) is fused
host-side into one [128, 6, 1280] tensor and fetched with just six
per-kc DMAs on the sync ring.  Block 0 runs kc-outer so matmul pass k
only needs head chunk k -- the PE streams gaplessly from the first
chunk's arrival instead of waiting for the full weight matrix.  All
input DMAs ride the sync ring in consumption order (single FIFO queue
=> bytes arrive in priority order); output DMAs ride the scalar ring.
"""

import os

import numpy as np
import ml_dtypes

import concourse.bass as bass
import concourse.mybir as mybir
import concourse.tile as tile
from concourse import bacc
from concourse.bass_utils import run_bass_kernel_spmd

P = 128
C = 768
KC = C // P            # 6 contraction chunks
N_CORES = 8
B, H, W = 16, 56, 56
ROWS = B * H * W       # 50176
RPC = ROWS // N_CORES  # 6272 rows per core
TT = RPC // P          # 49 token tiles per core
TBLK = 4               # token tiles per input DMA block (512 tokens)
HBLK = 4               # token tiles in the fused head block
HW_ = C + HBLK * P     # head chunk cols per kc: 768 w + 512 x0
N_WARM = 14            # PE pre-warm matmuls issued during the DMA head
OUT_BF16 = os.environ.get("GWTA_OUT", "bf16") == "bf16"

# Internal matmul dtype: bf16 halves input DMA and streams 1 col/cycle.
VARIANT = os.environ.get("GWTA_VARIANT", "bf16")

LAST_STATS: dict = {}

_IN_DT = {
    "bf16": mybir.dt.bfloat16,
    "fp32r": mybir.dt.float32r,
    "fp32": mybir.dt.float32,
}


def _build_nc(variant: str) -> bass.Bass:
    in_dt = _IN_DT[variant]
    out_dt = mybir.dt.bfloat16 if OUT_BF16 else mybir.dt.float32
    nc = bacc.Bacc(None, target_bir_lowering=False)
    hx = nc.declare_dram_parameter("hx", [P, KC, HW_], in_dt, isOutput=False)
    xT = nc.declare_dram_parameter("xT", [C, RPC], in_dt, isOutput=False)
    b = nc.declare_dram_parameter("b", [P, C], mybir.dt.float32, isOutput=False)
    out = nc.declare_dram_parameter("out", [RPC, C], out_dt, isOutput=True)

    with tile.TileContext(nc) as tc:
        with (
            tc.tile_pool(name="const", bufs=1) as const,
            tc.tile_pool(name="xp", bufs=3) as xp,
            tc.tile_pool(name="op", bufs=4) as op,
            tc.tile_pool(name="pp", bufs=1, space="PSUM") as pp,
        ):
            # PE pre-warm: matmuls on zeroed SBUF keep the PE busy from the
            # end of the NEFF preamble (~6.5us) until the first head chunk
            # lands (~9.7us), so the HAM activity window accumulates
            # continuous busy time and un-throttles to 2.4GHz shortly after
            # the real stream begins.  The memset is small so the first
            # warm matmul issues as early as possible.
            g_rhs = const.tile([P, 256], in_dt)
            nc.gpsimd.memset(g_rhs[:], 0.0)
            warm = pp.tile([P, C], mybir.dt.float32, tag="pt3")
            for _ in range(N_WARM):
                nc.tensor.matmul(
                    warm[:, 0:256], g_rhs[:, 0:P], g_rhs[:],
                    start=True, stop=True,
                )

            xTr = xT.rearrange("(kc p) t -> p kc t", p=P)

            # Head chunks: [w_kc | x0_kc] fused, one DMA per kc on the sync
            # ring.  Pass kc of block 0 needs only chunk kc.
            ht = const.tile([P, KC, HW_], in_dt, name="ht")
            for kc in range(KC):
                nc.sync.dma_start(out=ht[:, kc, :], in_=hx[:, kc, :])
            # Bias (first read by DVE ~15us), then 2 prefetched x blocks --
            # all behind the head chunks on the same FIFO queue.
            bt = const.tile([P, C], mybir.dt.float32)
            nc.sync.dma_start(out=bt[:], in_=b[:])

            blocks = [TBLK] * ((TT - HBLK) // TBLK) + [1]
            assert HBLK + sum(blocks) == TT

            xbts: dict = {}

            def issue_xb(j: int) -> None:
                nbj = blocks[j]
                t0j = HBLK + sum(blocks[:j])
                xt = xp.tile([P, KC, TBLK * P], in_dt, tag="xb", name="xb")
                nc.sync.dma_start(
                    out=xt[:, :, : nbj * P],
                    in_=xTr[:, :, t0j * P : (t0j + nbj) * P],
                )
                xbts[j] = xt

            issue_xb(0)
            issue_xb(1)

            def wslice(kc, half):
                return (
                    ht[:, kc, 0:512] if half == 0 else ht[:, kc, 512:C]
                )

            def evict(g: int, pt) -> None:
                ot = op.tile([P, C], out_dt, tag="ot")
                # split at the PSUM bank boundary (one bank per DVE read)
                nc.vector.tensor_add(
                    out=ot[:, 0:512], in0=pt[:, 0:512], in1=bt[:, 0:512]
                )
                nc.vector.tensor_add(
                    out=ot[:, 512:C], in0=pt[:, 512:C], in1=bt[:, 512:C]
                )
                if g == TT - 1:
                    # Final tile: split the store across both HWDGE rings so
                    # the two halves transfer (and pay HBM receipt) in
                    # parallel on the tail critical path.
                    nc.scalar.dma_start(
                        out=out[g * P : (g + 1) * P, 0:512], in_=ot[:, 0:512]
                    )
                    nc.sync.dma_start(
                        out=out[g * P : (g + 1) * P, 512:C], in_=ot[:, 512:C]
                    )
                else:
                    nc.scalar.dma_start(
                        out=out[g * P : (g + 1) * P, :], in_=ot[:]
                    )

            # Block 0: kc-outer so pass kc only needs head chunk kc.  The
            # kc=5 pass runs s-ascending, so psum tile s=0 completes first
            # and its eviction clears tag pt0 just before block 1 reuses it.
            pts = [
                pp.tile([P, C], mybir.dt.float32, tag=f"pt{s}", name=f"pt{s}")
                for s in range(HBLK)
            ]
            for kc in range(KC):
                for s in range(HBLK):
                    lhsT = ht[:, kc, C + s * P : C + (s + 1) * P]
                    nc.tensor.matmul(
                        pts[s][:, 0:512], lhsT, wslice(kc, 0),
                        start=(kc == 0), stop=(kc == KC - 1),
                    )
                    nc.tensor.matmul(
                        pts[s][:, 512:C], lhsT, wslice(kc, 1),
                        start=(kc == 0), stop=(kc == KC - 1),
                    )
            for s in range(HBLK):
                evict(s, pts[s])

            # Blocks 1..: s-outer, one 4-tile input DMA each, prefetch
            # distance 2.
            g0 = HBLK
            for bi, nb in enumerate(blocks):
                if bi + 2 < len(blocks):
                    issue_xb(bi + 2)
                xt = xbts.pop(bi)
                # Tag rotation offset +3: block 1's first tile reuses the
                # warm-matmul tag pt3 (long free) instead of pt0, whose
                # block-0 eviction lands only ~0.3us earlier.
                pts = [
                    pp.tile(
                        [P, C], mybir.dt.float32,
                        tag=f"pt{(g0 + s + 3) % 4}",
                        name=f"pt{(g0 + s + 3) % 4}",
                    )
                    for s in range(nb)
                ]
                for s in range(nb):
                    for kc in range(KC):
                        lhsT = xt[:, kc, s * P : (s + 1) * P]
                        nc.tensor.matmul(
                            pts[s][:, 0:512], lhsT, wslice(kc, 0),
                            start=(kc == 0), stop=(kc == KC - 1),
                        )
                        nc.tensor.matmul(
                            pts[s][:, 512:C], lhsT, wslice(kc, 1),
                            start=(kc == 0), stop=(kc == KC - 1),
                        )
                for s in range(nb):
                    evict(g0 + s, pts[s])
                g0 += nb
    nc.compile()
    return nc


def _fold_weights(qkv_w, qkv_b, proj_w, proj_b, pe):
    v_w = qkv_w[2 * 4 : 3 * 4].astype(np.float64)   # [4, 4]
    v_b = qkv_b[2 * 4 : 3 * 4].astype(np.float64)   # [4]
    bd = np.kron(np.eye(C // 4), v_w.T)             # y_flat @ bd == groupwise v
    w_eff = bd @ proj_w.astype(np.float64).T        # [768, 768]
    b_eff = (
        np.tile(v_b, C // 4) @ proj_w.astype(np.float64).T
        + proj_b.astype(np.float64)
        + pe[:C].astype(np.float64) @ w_eff
    )
    return w_eff, b_eff


def _enable_tracing_shims():
    """Dev-only (GWTA_TRACE=1): restore the NTFF profile hook that this
    image's `antenv` is missing, and keep trace artifacts local instead of
    uploading.  Never active when the kernel is called normally."""
    import sys
    import types

    try:
        from antenv import axon_hooks  # noqa: F401
    except ImportError:
        import antenv
        from trn_agent_boot.trn_boot import _ntff_profile_via_ctypes

        mod = types.ModuleType("antenv.axon_hooks")
        mod._hook = _ntff_profile_via_ctypes("/opt/axon/libaxon_pjrt.so")
        mod.get_axon_ntff_profile_hook = lambda: mod._hook
        mod.set_axon_ntff_profile_hook = lambda h: setattr(mod, "_hook", h)
        sys.modules["antenv.axon_hooks"] = mod
        antenv.axon_hooks = mod

    import concourse.bass_utils as bu

    bu.upload_artifacts = lambda tmpdir: f"local:{tmpdir}"


def kernel(x, qkv_w, qkv_b, proj_w, proj_b, pe):
    x = np.asarray(x, np.float32)
    w_eff, b_eff = _fold_weights(
        np.asarray(qkv_w), np.asarray(qkv_b),
        np.asarray(proj_w), np.asarray(proj_b), np.asarray(pe),
    )

    variant = VARIANT
    if variant == "bf16":
        cast = lambda a: np.ascontiguousarray(a, dtype=ml_dtypes.bfloat16)
    else:
        cast = lambda a: np.ascontiguousarray(a, dtype=np.float32)

    w_dev = cast(w_eff)                       # [768, 768]
    w_chunks = np.asarray(w_dev).reshape(KC, P, C)  # [kc, p, c]
    b_dev = np.broadcast_to(b_eff.astype(np.float32), (P, C)).copy()

    x_flat = x.reshape(ROWS, C)
    in_maps = []
    for c in range(N_CORES):
        xc = x_flat[c * RPC : (c + 1) * RPC]
        xt = cast(xc.T)                       # [768, 6272]
        xt_np = np.asarray(xt)
        # Fused head: [p, kc, 768 w | 512 x0]
        hx = np.empty((P, KC, HW_), dtype=xt_np.dtype)
        hx[:, :, :C] = w_chunks.transpose(1, 0, 2)
        hx[:, :, C:] = (
            xt_np[:, : HBLK * P].reshape(KC, P, HBLK * P).transpose(1, 0, 2)
        )
        in_maps.append({"hx": hx, "xT": xt_np, "b": b_dev})

    nc = _build_nc(variant)
    trace = bool(int(os.environ.get("GWTA_TRACE", "0")))
    kw = {}
    if trace:
        _enable_tracing_shims()
        kw["tmpdir"] = os.environ.get("GWTA_TRACE_DIR") or None
    r = run_bass_kernel_spmd(nc, in_maps, list(range(N_CORES)), trace=trace, **kw)

    LAST_STATS.clear()
    LAST_STATS.update(
        exec_time_ns=r.exec_time_ns,
        mean_exec_time_ns=r.mean_exec_time_ns,
        variant=variant,
    )

    out = np.empty((ROWS, C), np.float32)
    for c in range(N_CORES):
        out[c * RPC : (c + 1) * RPC] = np.asarray(
            r.results[c]["out"]
        ).astype(np.float32)
    return out.reshape(B, H, W, C)
